# revision 11
# baseline (speedup 1.0000x reference)
"""Biaffine kernel for Trainium2, data-parallel over batch on 8 NeuronCores.

Problem: inputs [8,512,768] f32, weight1 [768,12,768], weight2 [1537,12],
mask [8,512] i32 -> logits [8,12,512,512] f32 (see reference).

Key trick: masked outputs are -1e12 (or -2e12), and f32 addition absorbs any
|v| < half-ulp(1e12) = 32768. Raw logits are |v| <~ 1e3, so we never mask the
matmul inputs: out = raw + C with C in {0, -1e12, -2e12} reproduces the
reference bit-exactly on masked entries and exactly on unmasked ones.

Per core (batch b):
  XT[i,x]    = X[x,i]                             (PE transpose)
  tmpT[j,x]  = sum_i W1[i,o,j] * XT[i,x]          (matmul1, per o)
  raw[x,y]   = sum_j tmpT[j,x] * XT[j,y]          (matmul2, per o)
               + linjT[o,y]                       (K=24 selector matmul)
  out[o,x,y] = (raw + linT[o,x]) + C[x,y]         (one DVE scalar_tensor_tensor)
where linT[o,x] = lin_i[x,o]+bias[o], linjT[o,y] = lin_j[y,o],
C = NEG*(m outer m) + C0,  C0 = -NEG*tril(k=-1) - NEG.
"""

import numpy as np

import concourse.bass as bass
import concourse.mybir as mybir
import concourse.tile as tile
from concourse import bacc
from concourse.bass_utils import run_bass_kernel_spmd

B, L, H, O = 8, 512, 768, 12
NEG = 1e12
F32 = mybir.dt.float32
F32R = mybir.dt.float32r
NCORES = 8

_cached_nc = None


def build_nc():
    nc = bacc.Bacc(None, target_bir_lowering=False)

    x_d = nc.dram_tensor("x", [L, H], F32R, kind="ExternalInput")
    w1_d = nc.dram_tensor("w1", [H, O, H], F32R, kind="ExternalInput")
    w2a_d = nc.dram_tensor("w2a", [H + 1, 128], F32R, kind="ExternalInput")
    selo_d = nc.dram_tensor("selo", [128, O * 128], F32R, kind="ExternalInput")
    mrow_d = nc.dram_tensor("mrow", [1, L], F32R, kind="ExternalInput")
    ones_d = nc.dram_tensor("ones1", [1, L], F32R, kind="ExternalInput")
    ident_d = nc.dram_tensor("ident", [128, 128], F32R, kind="ExternalInput")
    c0_d = nc.dram_tensor("c0", [L, L], F32, kind="ExternalInput")
    out_d = nc.dram_tensor("out", [O, L, L], F32, kind="ExternalOutput")

    KT = H // 128   # 6 k-tiles over i/j
    XC = L // 128   # 4 x-chunks

    with tile.TileContext(nc) as tc:
        with (
            tc.tile_pool(name="const", bufs=1) as cpool,
            tc.tile_pool(name="work", bufs=1) as wpool,
            tc.tile_pool(name="w1p", bufs=2) as w1pool,
            tc.tile_pool(name="tmpp", bufs=2) as tmppool,
            tc.tile_pool(name="outp", bufs=4) as outpool,
        ):
            # ident+X first on the sync queue (transposes gate everything);
            # W1[o=0] prefetch goes on the scalar HWDGE queue so it does not
            # delay the transposes' DMA waits.
            ident = cpool.tile([128, 128], F32R, tag="ident")
            nc.sync.dma_start(ident[:], ident_d[:])
            xnat = wpool.tile([128, XC * H], F32R, tag="xnat")
            nc.sync.dma_start(
                xnat[:].rearrange("p (c i) -> p c i", c=XC),
                x_d[:].rearrange("(c p) i -> p c i", p=128),
            )
            w1t_next = w1pool.tile([128, KT * H], F32R, tag="w1t")
            nc.scalar.dma_start(
                w1t_next[:].rearrange("p (k i) -> p k i", k=KT),
                w1_d[:, 0, :].rearrange("(k p) i -> p k i", p=128),
            )
            mrow = cpool.tile([1, L], F32R, tag="mrow")
            nc.sync.dma_start(mrow[:], mrow_d[:])
            ones1 = cpool.tile([1, L], F32R, tag="ones1")
            nc.sync.dma_start(ones1[:], ones_d[:])
            w2sb = cpool.tile([128, KT * 128], F32R, tag="w2sb")
            for kt in range(KT):
                nc.sync.dma_start(
                    w2sb[:, kt * 128 : (kt + 1) * 128],
                    w2a_d[kt * 128 : (kt + 1) * 128, :],
                )
            w2last = cpool.tile([1, 128], F32R, tag="w2last")
            nc.sync.dma_start(w2last[:], w2a_d[H : H + 1, :])
            selo = cpool.tile([128, O * 128], F32R, tag="selo")
            nc.scalar.dma_start(selo[:], selo_d[:])
            csb = cpool.tile([128, XC * L], F32, tag="csb")
            nc.scalar.dma_start(
                csb[:].rearrange("p (c y) -> p c y", c=XC),
                c0_d[:].rearrange("(c p) y -> p c y", p=128),
            )
            xt = wpool.tile([128, KT * L], F32R, tag="xt")
            augall = wpool.tile([128, L], F32R, tag="augall")
            linTT = wpool.tile([128, XC * O], F32, tag="linTT")
            with tc.tile_pool(name="pspro", bufs=1, space="PSUM") as pspro:
                for c in range(XC):
                    for kt in range(KT):
                        tp = pspro.tile([128, 128], F32R, tag="tp", bufs=2)
                        nc.tensor.transpose(
                            tp[:],
                            xnat[:, c * H + kt * 128 : c * H + (kt + 1) * 128],
                            ident[:],
                        )
                        nc.vector.tensor_copy(
                            xt[:, kt * L + c * 128 : kt * L + (c + 1) * 128], tp[:]
                        )

                # --- C map: csb <- NEG * (mx outer my) + csb ---
                for c in range(XC):
                    pm = pspro.tile([128, L], F32, tag="pm", bufs=2)
                    nc.tensor.matmul(
                        pm[:],
                        mrow[:, c * 128 : (c + 1) * 128],
                        mrow[:],
                        start=True,
                        stop=True,
                    )
                    nc.vector.scalar_tensor_tensor(
                        out=csb[:, c * L : (c + 1) * L],
                        in0=pm[:],
                        scalar=NEG,
                        in1=csb[:, c * L : (c + 1) * L],
                        op0=mybir.AluOpType.mult,
                        op1=mybir.AluOpType.add,
                    )

                # --- augall [24, 512]: rows o = linT'[o], rows 12+o = linjT'[o]
                pa = pspro.tile([128, L], F32, tag="pa", bufs=1)
                for kt in range(KT):
                    nc.tensor.matmul(
                        pa[:],
                        w2sb[:, kt * 128 : (kt + 1) * 128],
                        xt[:, kt * L : (kt + 1) * L],
                        start=(kt == 0),
                        stop=False,
                    )
                nc.tensor.matmul(
                    pa[:], w2last[:], ones1[:], start=False, stop=True
                )
                nc.vector.tensor_copy(augall[:], pa[:])

                # --- linTT [128, XC*O]: transpose of augall rows 0..11 ---
                for c in range(XC):
                    pt = pspro.tile([128, O], F32R, tag="pt", bufs=2)
                    nc.tensor.transpose(
                        pt[:],
                        augall[0:O, c * 128 : (c + 1) * 128],
                        ident[0:O, 0:O],
                    )
                    nc.vector.tensor_copy(linTT[:, c * O : (c + 1) * O], pt[:])

            # --- main loop over labels ---
            with tc.tile_pool(name="psmain", bufs=1, space="PSUM") as psm:
                for o in range(O):
                    w1t = w1t_next
                    if o + 1 < O:
                        w1t_next = w1pool.tile([128, KT * H], F32R, tag="w1t")
                        nc.sync.dma_start(
                            w1t_next[:].rearrange("p (k i) -> p k i", k=KT),
                            w1_d[:, o + 1, :].rearrange("(k p) i -> p k i", p=128),
                        )

                    # matmul1: tmpT[j, x] (6 m-chunks x 6 k-tiles)
                    tmp = tmppool.tile([128, KT * L], F32R, tag="tmp")
                    for m in range(KT):
                        p1 = psm.tile([128, L], F32, tag="t1", bufs=3)
                        for kt in range(KT):
                            nc.tensor.matmul(
                                p1[:],
                                w1t[:, kt * H + m * 128 : kt * H + (m + 1) * 128],
                                xt[:, kt * L : (kt + 1) * L],
                                start=(kt == 0),
                                stop=(kt == KT - 1),
                            )
                        nc.vector.tensor_copy(tmp[:, m * L : (m + 1) * L], p1[:])

                    # matmul2 + linj aug + epilogue per x-chunk
                    for c in range(XC):
                        p2 = psm.tile([128, L], F32, tag="t2", bufs=5)
                        for jr in range(KT):
                            nc.tensor.matmul(
                                p2[:],
                                tmp[:, jr * L + c * 128 : jr * L + (c + 1) * 128],
                                xt[:, jr * L : (jr + 1) * L],
                                start=(jr == 0),
                                stop=False,
                            )
                        nc.tensor.matmul(
                            p2[:],
                            selo[:, o * 128 : (o + 1) * 128],
                            augall[:],
                            start=False,
                            stop=True,
                        )
                        osb = outpool.tile([128, L], F32, tag="osb")
                        nc.vector.scalar_tensor_tensor(
                            out=osb[:],
                            in0=p2[:],
                            scalar=linTT[:, c * O + o : c * O + o + 1],
                            in1=csb[:, c * L : (c + 1) * L],
                            op0=mybir.AluOpType.add,
                            op1=mybir.AluOpType.add,
                        )
                        nc.scalar.dma_start(
                            out_d[o, c * 128 : (c + 1) * 128, :], osb[:]
                        )

    nc.compile()
    return nc


def _get_nc():
    global _cached_nc
    if _cached_nc is None:
        _cached_nc = build_nc()
    return _cached_nc


def _host_consts(weight2):
    w2a = np.zeros((H + 1, 128), dtype=np.float32)
    # cols o: linT' = lin_i + bias; cols O+o: linjT' = lin_j
    w2a[:H, :O] = weight2[:H, :]
    w2a[H, :O] = weight2[2 * H, :]
    w2a[:H, O : 2 * O] = weight2[H : 2 * H, :]
    selo = np.zeros((128, O * 128), dtype=np.float32)
    for o in range(O):
        selo[O + o, o * 128 : (o + 1) * 128] = 1.0
    ident = np.eye(128, dtype=np.float32)
    ones1 = np.ones((1, L), dtype=np.float32)
    tril = np.tril(np.ones((L, L), dtype=np.float32), k=-1)
    c0 = (-NEG * tril - NEG).astype(np.float32)
    return w2a, selo, ident, ones1, c0


def _run(inputs, weight1, weight2, mask, trace=False):
    nc = _get_nc()
    w2a, selo, ident, ones1, c0 = _host_consts(np.asarray(weight2, dtype=np.float32))
    w1 = np.ascontiguousarray(np.asarray(weight1, dtype=np.float32))
    in_maps = []
    for b in range(NCORES):
        m = np.asarray(mask[b], dtype=np.float32)
        in_maps.append(
            {
                "x": np.ascontiguousarray(np.asarray(inputs[b], dtype=np.float32)),
                "w1": w1,
                "w2a": w2a,
                "selo": selo,
                "mrow": np.ascontiguousarray(m[None, :]),
                "ones1": ones1,
                "ident": ident,
                "c0": c0,
            }
        )
    br = run_bass_kernel_spmd(nc, in_maps, core_ids=list(range(NCORES)), trace=trace)
    out = np.stack([br.results[b]["out"] for b in range(NCORES)], axis=0)
    return out, br


def kernel(inputs, weight1, weight2, mask):
    out, _ = _run(inputs, weight1, weight2, mask)
    return out


# revision 12
# speedup vs baseline: 1.0337x; 1.0337x over previous
"""Biaffine kernel for Trainium2, data-parallel over batch on 8 NeuronCores.

Problem: inputs [8,512,768] f32, weight1 [768,12,768], weight2 [1537,12],
mask [8,512] i32 -> logits [8,12,512,512] f32 (see reference).

Key trick: masked outputs are -1e12 (or -2e12), and f32 addition absorbs any
|v| < half-ulp(1e12) = 32768. Raw logits are |v| <~ 1e3, so we never mask the
matmul inputs: out = raw + C with C in {0, -1e12, -2e12} reproduces the
reference bit-exactly on masked entries and exactly on unmasked ones.

Per core (batch b):
  XT[i,x]    = X[x,i]                             (PE transpose)
  tmpT[j,x]  = sum_i W1[i,o,j] * XT[i,x]          (matmul1, per o)
  raw[x,y]   = sum_j tmpT[j,x] * XT[j,y]          (matmul2, per o)
               + linjT[o,y]                       (K=24 selector matmul)
  out[o,x,y] = (raw + linT[o,x]) + C[x,y]         (one DVE scalar_tensor_tensor)
where linT[o,x] = lin_i[x,o]+bias[o], linjT[o,y] = lin_j[y,o],
C = NEG*(m outer m) + C0,  C0 = -NEG*tril(k=-1) - NEG.
"""

import numpy as np

import concourse.bass as bass
import concourse.mybir as mybir
import concourse.tile as tile
from concourse import bacc
from concourse.bass_utils import run_bass_kernel_spmd

B, L, H, O = 8, 512, 768, 12
NEG = 1e12
F32 = mybir.dt.float32
F32R = mybir.dt.float32r
NCORES = 8

_cached_nc = None


def build_nc():
    nc = bacc.Bacc(None, target_bir_lowering=False)

    x_d = nc.dram_tensor("x", [L, H], F32R, kind="ExternalInput")
    w1_d = nc.dram_tensor("w1", [H, O, H], F32R, kind="ExternalInput")
    w2a_d = nc.dram_tensor("w2a", [H + 1, 128], F32R, kind="ExternalInput")
    selo_d = nc.dram_tensor("selo", [128, O * 128], F32R, kind="ExternalInput")
    mrow_d = nc.dram_tensor("mrow", [1, L], F32R, kind="ExternalInput")
    ones_d = nc.dram_tensor("ones1", [1, L], F32R, kind="ExternalInput")
    ident_d = nc.dram_tensor("ident", [128, 128], F32R, kind="ExternalInput")
    c0_d = nc.dram_tensor("c0", [L, L], F32, kind="ExternalInput")
    out_d = nc.dram_tensor("out", [O, L, L], F32, kind="ExternalOutput")

    KT = H // 128   # 6 k-tiles over i/j
    XC = L // 128   # 4 x-chunks

    with tile.TileContext(nc) as tc:
        with (
            tc.tile_pool(name="const", bufs=1) as cpool,
            tc.tile_pool(name="work", bufs=1) as wpool,
            tc.tile_pool(name="w1p", bufs=2) as w1pool,
            tc.tile_pool(name="tmpp", bufs=2) as tmppool,
            tc.tile_pool(name="outp", bufs=4) as outpool,
        ):
            # ident+X first on the sync queue (transposes gate everything);
            # W1[o=0] prefetch goes on the scalar HWDGE queue so it does not
            # delay the transposes' DMA waits.
            ident = cpool.tile([128, 128], F32R, tag="ident")
            nc.sync.dma_start(ident[:], ident_d[:])
            xnat = wpool.tile([128, XC * H], F32R, tag="xnat")
            for c in range(XC):
                nc.sync.dma_start(
                    xnat[:, c * H : (c + 1) * H], x_d[c * 128 : (c + 1) * 128, :]
                )
            w1t_next = w1pool.tile([128, KT * H], F32R, tag="w1t")
            for kt in range(KT):
                nc.scalar.dma_start(
                    w1t_next[:, kt * H : (kt + 1) * H],
                    w1_d[kt * 128 : (kt + 1) * 128, 0, :],
                )
            mrow = cpool.tile([1, L], F32R, tag="mrow")
            nc.sync.dma_start(mrow[:], mrow_d[:])
            ones1 = cpool.tile([1, L], F32R, tag="ones1")
            nc.sync.dma_start(ones1[:], ones_d[:])
            w2sb = cpool.tile([128, KT * 128], F32R, tag="w2sb")
            for kt in range(KT):
                nc.sync.dma_start(
                    w2sb[:, kt * 128 : (kt + 1) * 128],
                    w2a_d[kt * 128 : (kt + 1) * 128, :],
                )
            w2last = cpool.tile([1, 128], F32R, tag="w2last")
            nc.sync.dma_start(w2last[:], w2a_d[H : H + 1, :])
            selo = cpool.tile([128, O * 128], F32R, tag="selo")
            nc.scalar.dma_start(selo[:], selo_d[:])
            csb = cpool.tile([128, XC * L], F32, tag="csb")
            for c in range(XC):
                nc.scalar.dma_start(
                    csb[:, c * L : (c + 1) * L], c0_d[c * 128 : (c + 1) * 128, :]
                )
            xt = wpool.tile([128, KT * L], F32R, tag="xt")
            augall = wpool.tile([128, L], F32R, tag="augall")
            linTT = wpool.tile([128, XC * O], F32, tag="linTT")
            with tc.tile_pool(name="pspro", bufs=1, space="PSUM") as pspro:
                for c in range(XC):
                    for kt in range(KT):
                        tp = pspro.tile([128, 128], F32R, tag="tp", bufs=2)
                        nc.tensor.transpose(
                            tp[:],
                            xnat[:, c * H + kt * 128 : c * H + (kt + 1) * 128],
                            ident[:],
                        )
                        nc.vector.tensor_copy(
                            xt[:, kt * L + c * 128 : kt * L + (c + 1) * 128], tp[:]
                        )

                # --- C map: csb <- NEG * (mx outer my) + csb ---
                for c in range(XC):
                    pm = pspro.tile([128, L], F32, tag="pm", bufs=2)
                    nc.tensor.matmul(
                        pm[:],
                        mrow[:, c * 128 : (c + 1) * 128],
                        mrow[:],
                        start=True,
                        stop=True,
                    )
                    nc.vector.scalar_tensor_tensor(
                        out=csb[:, c * L : (c + 1) * L],
                        in0=pm[:],
                        scalar=NEG,
                        in1=csb[:, c * L : (c + 1) * L],
                        op0=mybir.AluOpType.mult,
                        op1=mybir.AluOpType.add,
                    )

                # --- augall [24, 512]: rows o = linT'[o], rows 12+o = linjT'[o]
                pa = pspro.tile([128, L], F32, tag="pa", bufs=1)
                for kt in range(KT):
                    nc.tensor.matmul(
                        pa[:],
                        w2sb[:, kt * 128 : (kt + 1) * 128],
                        xt[:, kt * L : (kt + 1) * L],
                        start=(kt == 0),
                        stop=False,
                    )
                nc.tensor.matmul(
                    pa[:], w2last[:], ones1[:], start=False, stop=True
                )
                nc.vector.tensor_copy(augall[:], pa[:])

                # --- linTT [128, XC*O]: transpose of augall rows 0..11 ---
                for c in range(XC):
                    pt = pspro.tile([128, O], F32R, tag="pt", bufs=2)
                    nc.tensor.transpose(
                        pt[:],
                        augall[0:O, c * 128 : (c + 1) * 128],
                        ident[0:O, 0:O],
                    )
                    nc.vector.tensor_copy(linTT[:, c * O : (c + 1) * O], pt[:])

            # --- main loop over labels ---
            with tc.tile_pool(name="psmain", bufs=1, space="PSUM") as psm:
                for o in range(O):
                    w1t = w1t_next
                    if o + 1 < O:
                        w1t_next = w1pool.tile([128, KT * H], F32R, tag="w1t")
                        for kt in range(KT):
                            nc.sync.dma_start(
                                w1t_next[:, kt * H : (kt + 1) * H],
                                w1_d[kt * 128 : (kt + 1) * 128, o + 1, :],
                            )

                    # matmul1: tmpT[j, x] (6 m-chunks x 6 k-tiles)
                    tmp = tmppool.tile([128, KT * L], F32R, tag="tmp")
                    for m in range(KT):
                        p1 = psm.tile([128, L], F32, tag="t1", bufs=3)
                        for kt in range(KT):
                            nc.tensor.matmul(
                                p1[:],
                                w1t[:, kt * H + m * 128 : kt * H + (m + 1) * 128],
                                xt[:, kt * L : (kt + 1) * L],
                                start=(kt == 0),
                                stop=(kt == KT - 1),
                            )
                        nc.vector.tensor_copy(tmp[:, m * L : (m + 1) * L], p1[:])

                    # matmul2 + linj aug + epilogue per x-chunk
                    for c in range(XC):
                        p2 = psm.tile([128, L], F32, tag="t2", bufs=5)
                        for jr in range(KT):
                            nc.tensor.matmul(
                                p2[:],
                                tmp[:, jr * L + c * 128 : jr * L + (c + 1) * 128],
                                xt[:, jr * L : (jr + 1) * L],
                                start=(jr == 0),
                                stop=False,
                            )
                        nc.tensor.matmul(
                            p2[:],
                            selo[:, o * 128 : (o + 1) * 128],
                            augall[:],
                            start=False,
                            stop=True,
                        )
                        osb = outpool.tile([128, L], F32, tag="osb")
                        nc.vector.scalar_tensor_tensor(
                            out=osb[:],
                            in0=p2[:],
                            scalar=linTT[:, c * O + o : c * O + o + 1],
                            in1=csb[:, c * L : (c + 1) * L],
                            op0=mybir.AluOpType.add,
                            op1=mybir.AluOpType.add,
                        )
                        nc.scalar.dma_start(
                            out_d[o, c * 128 : (c + 1) * 128, :], osb[:]
                        )

    nc.compile()
    return nc


def _get_nc():
    global _cached_nc
    if _cached_nc is None:
        _cached_nc = build_nc()
    return _cached_nc


def _host_consts(weight2):
    w2a = np.zeros((H + 1, 128), dtype=np.float32)
    # cols o: linT' = lin_i + bias; cols O+o: linjT' = lin_j
    w2a[:H, :O] = weight2[:H, :]
    w2a[H, :O] = weight2[2 * H, :]
    w2a[:H, O : 2 * O] = weight2[H : 2 * H, :]
    selo = np.zeros((128, O * 128), dtype=np.float32)
    for o in range(O):
        selo[O + o, o * 128 : (o + 1) * 128] = 1.0
    ident = np.eye(128, dtype=np.float32)
    ones1 = np.ones((1, L), dtype=np.float32)
    tril = np.tril(np.ones((L, L), dtype=np.float32), k=-1)
    c0 = (-NEG * tril - NEG).astype(np.float32)
    return w2a, selo, ident, ones1, c0


def _run(inputs, weight1, weight2, mask, trace=False):
    nc = _get_nc()
    w2a, selo, ident, ones1, c0 = _host_consts(np.asarray(weight2, dtype=np.float32))
    w1 = np.ascontiguousarray(np.asarray(weight1, dtype=np.float32))
    in_maps = []
    for b in range(NCORES):
        m = np.asarray(mask[b], dtype=np.float32)
        in_maps.append(
            {
                "x": np.ascontiguousarray(np.asarray(inputs[b], dtype=np.float32)),
                "w1": w1,
                "w2a": w2a,
                "selo": selo,
                "mrow": np.ascontiguousarray(m[None, :]),
                "ones1": ones1,
                "ident": ident,
                "c0": c0,
            }
        )
    br = run_bass_kernel_spmd(nc, in_maps, core_ids=list(range(NCORES)), trace=trace)
    out = np.stack([br.results[b]["out"] for b in range(NCORES)], axis=0)
    return out, br


def kernel(inputs, weight1, weight2, mask):
    out, _ = _run(inputs, weight1, weight2, mask)
    return out
